# revision 33
# baseline (speedup 1.0000x reference)
"""CVRP decoder kernel for 8 Trainium2 NeuronCores (pure batch data-parallel).

Self-contained: hardcodes shapes B=64,N=256,M=1000,S=500,E=128,H=8,D=16 and
shards the batch 8-per-core. Per core, per batch instance (software-pipelined
across instances, phase2(b-1) interleaved into the middle of phase1(b)):
  qT = WqT^T @ [eln; load]^T (PE, via PE-transposed eln), spread into a
    block-diagonal qtz so per-head scores come from ONE stacked K=128 matmul
    against the transposed kT (all operands at partition base 0 -- matmul
    operands at non-zero partition bases crash this stack at runtime)
  scoreT_h [l, n] (f32r, 1 col/cycle) -> ACT exp(score/4) straight out of
    PSUM -> DVE multiply by 0/1 mask (ACT exp of PE-transposed raw mask)
  att_h = v_aug^T @ u (bf16, output col-tile_position per head quad); the
    ones column of v_aug yields softmax denominators for free
  normalization: den rows extracted by a selection matmul, reciprocals
    broadcast across head rows by an indicator matmul, all in "att layout";
  the head-order permutation is folded into DMA-built permuted Wc^T halves
  final = cmb^T @ shk (shk is naturally transposed in DRAM); ACT tanh;
  DVE +mask; ACT exp(10x) with accum_out row sums; DVE normalize.

Environment workarounds: TileContext drain split onto single-wait NOPs and a
global one-wait-per-instruction legalization pass (this walrus rejects >1
sync wait per instruction).
"""

import re
from contextlib import ExitStack

import numpy as np

import concourse.bass as bass
import concourse.mybir as mybir
import concourse.tile as tile
from concourse.masks import make_identity

# ---------------------------------------------------------------- constants
B, N, M, S, E, H, D = 64, 256, 1000, 500, 128, 8, 16
SQRT_E = 11.313708498984761
CLIP = 10.0
NCORES = 8
BLOC = B // NCORES  # 8 batch instances per core

FP32 = mybir.dt.float32
F32R = mybir.dt.float32r
BF16 = mybir.dt.bfloat16
AF = mybir.ActivationFunctionType

# matmul dtype: float32r streams 1 col/cycle when free >= 256 (fp32 is 4)
MM_DT = F32R

L_TILES = [(i * 128, min(128, M - i * 128)) for i in range((M + 127) // 128)]
L2_TILES = [(i * 128, min(128, S - i * 128)) for i in range((S + 127) // 128)]


def _r(ap):
    """view an fp32 AP as the matmul dtype (bitwise; used on DRAM/DMA side)"""
    if MM_DT is FP32:
        return ap
    return ap.bitcast(MM_DT)


# ------------------------------------------------- tile drain-split patch
# This walrus build rejects >1 sync-wait on a Drain ("Too many sync wait
# commands"), so split the kernel-tail global-clock waits onto single-wait
# NOPs preceding the drain.
def _patch_tile_drain():
    from bass_rust import ScopedClock, VectorClock

    def _drain_and_barrier(self, tick_clock, wait_clock):
        gc = tick_clock.global_clock
        vals = [int(x) for x in re.findall(r"\d+", repr(gc))]
        for proc, tick in enumerate(vals):
            if tick > 0:
                partial = VectorClock()
                partial.require_at_least(proc, tick)
                nop = self.nc.sync.nop(nofuse=True, hint="split_drain_wait")
                wait_clock.add_sem_waits(nop.ins, ScopedClock({None: partial}))
        self.nc.sync.drain()  # waits covered by the NOPs above
        self.nc.all_engine_barrier()
        assert self.sems is not None
        popped = self.nc._tile_sem_poison_stack.pop()
        assert popped is self._sem_poison
        self.nc.clear_and_free_semaphores(list(self.sems.allocated().values()))
        self.nc.all_engine_barrier()

    tile.TileContext._drain_and_barrier = _drain_and_barrier


_patch_tile_drain()


def _legalize_single_waits(nc):
    """This walrus build accepts at most ONE sync-wait per instruction; hoist
    extra waits onto single-wait NOP carriers placed just before, on the same
    engine (engines execute in order, so the gate is preserved)."""
    n_multi_upd = 0
    for f in nc.m.functions:
        for bb in f.blocks:
            out = []
            for inst in bb.instructions:
                si = inst.sync_info
                if si is not None and len(si.on_wait) > 1:
                    waits = list(si.on_wait)
                    si.on_wait = waits[-1:]
                    for w in waits[:-1]:
                        nop = mybir.InstNoOp(
                            name=nc.get_next_instruction_name(), ins=[], outs=[])
                        nop.engine = inst.engine
                        nop.sync_info = mybir.SyncInfo(on_wait=[w], on_update=[])
                        out.append(nop)
                if si is not None and len(si.on_update) > 1:
                    n_multi_upd += 1
                out.append(inst)
            bb.instructions = out
    if n_multi_upd:
        print(f"WARNING: {n_multi_upd} instructions with >1 sync updates")


def build_nc(legalize=True):
    nc = bass.Bass(trn_type="TRN2", target_bir_lowering=False, debug=False)

    # DRAM I/O (per-core shard)
    eln = nc.dram_tensor("eln", [BLOC, N, E], FP32, kind="ExternalInput").ap()
    load = nc.dram_tensor("load", [BLOC, N], FP32, kind="ExternalInput").ap()
    solm = nc.dram_tensor("solm", [BLOC, N, S], FP32, kind="ExternalInput").ap()
    ninf = nc.dram_tensor("ninf", [BLOC, N, M], FP32, kind="ExternalInput").ap()
    k_in = nc.dram_tensor("k", [BLOC, H, M, D], FP32, kind="ExternalInput").ap()
    v_in = nc.dram_tensor("v", [BLOC, H, M, D], FP32, kind="ExternalInput").ap()
    ks_in = nc.dram_tensor("k_s", [BLOC, H, S, D], FP32, kind="ExternalInput").ap()
    vs_in = nc.dram_tensor("v_s", [BLOC, H, S, D], FP32, kind="ExternalInput").ap()
    shk = nc.dram_tensor("shk", [BLOC, E, M], FP32, kind="ExternalInput").ap()
    wq = nc.dram_tensor("wq", [E, E + 1], FP32, kind="ExternalInput").ap()
    wc = nc.dram_tensor("wc", [E, E], FP32, kind="ExternalInput").ap()
    out = nc.dram_tensor("out", [BLOC, N, M], FP32, kind="ExternalOutput").ap()

    with ExitStack() as ctx:
        tc = ctx.enter_context(tile.TileContext(nc))
        build_kernel(ctx, tc, eln, load, solm, ninf, k_in, v_in, ks_in, vs_in,
                     shk, wq, wc, out)
    if legalize:
        _legalize_single_waits(nc)
    return nc


def build_kernel(ctx, tc, eln, load, solm, ninf, k_in, v_in, ks_in, vs_in,
                 shk, wq, wc, out):
    nc = tc.nc
    ctx.enter_context(nc.allow_low_precision("fp32r rounding for PE matmuls"))

    # pools
    singles = ctx.enter_context(tc.tile_pool(name="singles", bufs=1))
    sb_in = ctx.enter_context(tc.tile_pool(name="sb_in", bufs=2))
    sb_k = ctx.enter_context(tc.tile_pool(name="sb_k", bufs=3))
    sb_u = ctx.enter_context(tc.tile_pool(name="sb_u", bufs=3))
    sb_misc = ctx.enter_context(tc.tile_pool(name="sb_misc", bufs=2))
    sb_big = ctx.enter_context(tc.tile_pool(name="sb_big", bufs=2))
    ps_score = ctx.enter_context(tc.tile_pool(name="ps_score", bufs=2, space="PSUM"))
    ps_att = ctx.enter_context(tc.tile_pool(name="ps_att", bufs=1, space="PSUM"))
    # two decoupled small-PSUM pools: MHA-loop transposes rotate separately
    # from the per-instance phase chain (elnT/qT/cmb/fs), so instance b's
    # tail never serializes behind instance b+1's loop tiles
    ps_loop = ctx.enter_context(tc.tile_pool(name="ps_loop", bufs=2, space="PSUM"))
    ps_phase = ctx.enter_context(tc.tile_pool(name="ps_phase", bufs=1, space="PSUM"))

    def loop_ps():
        return ps_loop.tile([128, 512], FP32, name="psl", tag="psl")

    def phase_ps():
        return ps_phase.tile([128, 512], FP32, name="psp", tag="psp")

    # ---------------- once-per-kernel prep ----------------
    ident = singles.tile([128, 128], FP32)
    make_identity(nc, ident)

    # Wq^T in head-spread layout: pack p in {0,1}; head h=4p+i sits at
    # partition rows 32i+d. wqT[i_dim, hd] = transpose(Wq[:, :128]).
    wq_sb = singles.tile([E, E + 1], FP32)
    nc.sync.dma_start(out=wq_sb, in_=wq)
    wqT_ps = phase_ps()
    nc.tensor.transpose(wqT_ps[:, 0:128], wq_sb[:, 0:128], ident)
    wqT = singles.tile([128, 128], MM_DT)
    nc.vector.tensor_copy(wqT, wqT_ps[:, 0:128])
    # Wq last column (the load weight), as a row vector in hd order
    wq_lc = singles.tile([1, 128], MM_DT)
    wq_col = bass.AP(tensor=wq.tensor, offset=wq.offset + E, ap=[[0, 1], [E + 1, E]])
    nc.sync.dma_start(out=wq_lc, in_=_r(wq_col))

    # Wc^T row-permuted to the attention PSUM layout: half c holds heads
    # 4c+i at rows 32i+d (rows 32i+16..31 zero). Built by strided DMA from wc.
    wcT_c = []
    for c in range(2):
        t = singles.tile([128, 128], MM_DT, name=f"wcT_c{c}", tag=f"wcT_c{c}")
        nc.vector.memset(t.bitcast(mybir.dt.int32), 0)
        for i in range(4):
            # rows 32i+d <- wc[:, 64c+16i+d]^T
            srcp = bass.AP(tensor=wc.tensor, offset=wc.offset + 64 * c + 16 * i,
                           ap=[[1, 16], [E, 128]])
            nc.sync.dma_start(out=t[32 * i:32 * i + 16, :], in_=_r(srcp))
        wcT_c.append(t)

    # persistent block-diagonal q tiles (zero blocks never rewritten)
    qtz_slots = [singles.tile([128, H * 256], MM_DT, name=f"qtz{i}",
                              tag=f"qtz{i}") for i in range(2)]
    for t in qtz_slots:
        nc.vector.memset(t.bitcast(mybir.dt.int32), 0)
    # persistent v tiles: cols 32h+16 = ones, 32h+[17:32) = zeros, set once
    vaug_slots = {}
    for tagp, n_s in (("m1", 3), ("m2", 3)):
        slots = []
        for i in range(n_s):
            t = singles.tile([128, H * 32], BF16, name=f"vaug_{tagp}{i}",
                             tag=f"vaug_{tagp}{i}")
            nc.vector.memset(t, 0.0)
            ones_ap = bass.AP(tensor=t.tensor, offset=t.offset + D,
                              ap=[[t.ap[0][0], 128], [32, H]])
            nc.vector.memset(ones_ap, 1.0)
            slots.append(t)
        vaug_slots[tagp] = slots

    # ---------------- per batch instance (software-pipelined) ----------------
    def prefetch(b):
        # issue instance b's input DMAs one instance ahead of use
        eln_sb = sb_misc.tile([128, 2, 128], FP32, tag="eln_sb")
        srce = bass.AP(tensor=eln.tensor, offset=eln.offset + b * N * E,
                       ap=[[E, 128], [128 * E, 2], [1, E]])
        nc.sync.dma_start(out=eln_sb, in_=srce)
        load_sb = sb_misc.tile([1, 256], MM_DT, tag="load_sb")
        load_row = bass.AP(tensor=load.tensor, offset=load.offset + b * N,
                           ap=[[0, 1], [1, N]])
        nc.sync.dma_start(out=load_sb, in_=_r(load_row))
        ninf2 = sb_big.tile([128, 2, M], FP32, tag="ninf2")
        srcn = bass.AP(tensor=ninf.tensor, offset=ninf.offset + b * N * M,
                       ap=[[M, 128], [128 * M, 2], [1, M]])
        nc.sync.dma_start(out=ninf2, in_=srcn)
        sol2 = sb_big.tile([128, 2, S], FP32, tag="sol2")
        srcs = bass.AP(tensor=solm.tensor, offset=solm.offset + b * N * S,
                       ap=[[S, 128], [128 * S, 2], [1, S]])
        nc.sync.dma_start(out=sol2, in_=srcs)
        return (eln_sb, load_sb, ninf2, sol2)

    def phase1(b, pf):
        eln_sb, load_sb, ninf2, sol2 = pf
        # ---- q projection
        elnT_ps = phase_ps()
        for nt in range(2):
            nc.tensor.transpose(elnT_ps[:, nt * 128:(nt + 1) * 128],
                                eln_sb[:, nt, :], ident)
        elnT = sb_misc.tile([128, 256], MM_DT, tag="elnT")
        nc.vector.tensor_copy(elnT, elnT_ps[:, 0:256])

        qT_ps = phase_ps()
        nc.tensor.matmul(qT_ps[:, 0:256], wqT, elnT, start=True, stop=False)
        nc.tensor.matmul(qT_ps[:, 0:256], wq_lc, load_sb, start=False, stop=True)
        qT = sb_misc.tile([128, 256], FP32, tag="qT")
        nc.vector.tensor_copy(qT, qT_ps[:, 0:256])
        qtz = qtz_slots[b % 2]
        for h in range(H):
            nc.sync.dma_start(out=qtz[16 * h:16 * h + 16, h * 256:(h + 1) * 256],
                              in_=_r(qT[16 * h:16 * h + 16, :]))

        # ---- masks (already prefetched)
        ninf_sb = [ninf2[:, 0, :], ninf2[:, 1, :]]
        sol_sb = [sol2[:, 0, :], sol2[:, 1, :]]

        mhc1 = mha_pass(tc, nc, b, k_in, v_in, L_TILES, M, qtz, ninf_sb,
                        sb_k, sb_u, ps_score, ps_att, loop_ps, sb_misc,
                        ident, vaug_slots, "m1")
        return (b, mhc1, ninf_sb, sol_sb)

    def phase1b(st):
        b, mhc1, ninf_sb, sol_sb = st
        mhc2 = mha_pass(tc, nc, b, ks_in, vs_in, L2_TILES, S, qtz_slots[b % 2],
                        sol_sb, sb_k, sb_u, ps_score, ps_att, loop_ps, sb_misc,
                        ident, vaug_slots, "m2")
        shk_sb = sb_big.tile([128, M], MM_DT, tag="shk_sb")
        nc.sync.dma_start(out=shk_sb, in_=_r(shk[b]))
        return (b, mhc1, mhc2, ninf_sb, shk_sb)

    def phase2(state):
        b, mhc1, mhc2, ninf_sb, shk_sb = state
        mh = sb_misc.tile([128, 512], MM_DT, tag="mh")
        nc.gpsimd.tensor_add(mh, mhc1, mhc2)

        # ---- combine (two halves accumulate over the full hd contraction)
        cmb_ps = phase_ps()
        for c in range(2):
            nc.tensor.matmul(cmb_ps[:, 0:256], wcT_c[c],
                             mh[:, c * 256:(c + 1) * 256],
                             start=(c == 0), stop=(c == 1))
        cmb = sb_misc.tile([128, 256], MM_DT, tag="cmb")
        nc.vector.tensor_copy(cmb, cmb_ps[:, 0:256])

        # ---- final scores, tanh, mask, softmax
        h2 = sb_big.tile([128, 2, M], FP32, tag="h2")
        for nt in range(2):
            t_sb = sb_big.tile([128, M], FP32, tag="t_sb")
            for mt2 in range(2):
                fs_ps = phase_ps()
                nc.tensor.matmul(fs_ps[:, 0:500],
                                 cmb[:, nt * 128:(nt + 1) * 128],
                                 shk_sb[:, mt2 * 500:(mt2 + 1) * 500],
                                 start=True, stop=True)
                nc.scalar.activation(t_sb[:, mt2 * 500:(mt2 + 1) * 500],
                                     fs_ps[:, 0:500], AF.Tanh,
                                     scale=float(1.0 / SQRT_E))
            # logits/10 = tanh + mask/10 (any large negative works after exp)
            nc.vector.tensor_add(t_sb, t_sb, ninf_sb[nt])
            h_sb = h2[:, nt, :]
            rowsum = sb_misc.tile([128, 1], FP32, tag="rowsum")
            nc.scalar.activation(h_sb, t_sb, AF.Exp, scale=float(CLIP),
                                 accum_out=rowsum)
            rs_r = sb_misc.tile([128, 1], FP32, tag="rs_r")
            nc.vector.reciprocal(rs_r, rowsum)
            nc.vector.tensor_scalar_mul(h_sb, h_sb, rs_r)
        dsto = bass.AP(tensor=out.tensor, offset=out.offset + b * N * M,
                       ap=[[M, 128], [128 * M, 2], [1, M]])
        nc.sync.dma_start(out=dsto, in_=h2)

    pf_cur = prefetch(0)
    pf_next = prefetch(1)
    state = phase1b(phase1(0, pf_cur))
    for b in range(1, BLOC):
        pf_cur, pf_next = pf_next, (prefetch(b + 1) if b + 1 < BLOC else None)
        half = phase1(b, pf_cur)
        phase2(state)
        state = phase1b(half)
    phase2(state)


def mha_pass(tc, nc, b, kd, vd, ltiles, LTOT, qtz, mask_sb, sb_k, sb_u,
             ps_score, ps_att, loop_ps, sb_misc, ident,
             vaug_slots, tagp):
    """One masked-MHA pass. Returns the normalized per-head attention output
    (att layout [128, 512]) ready to be summed and combined."""
    nlt = len(ltiles)
    att_all = ps_att.tile([128, 512], FP32, name="att", tag="att")
    att_ps = [att_all[:, 0:256], att_all[:, 256:512]]
    m01T = sb_u.tile([128, nlt, 256], BF16, name=f"m01T_{tagp}", tag=f"m01T_{tagp}")

    for lt, (l0, L) in enumerate(ltiles):
        # just-in-time mask transpose for PAIRS of l tiles: one exp covers
        # two tiles (tail-tile rows beyond L hold stale-but-finite data that
        # downstream slices never read)
        if lt % 2 == 0:
            mtp = loop_ps()
            for j in range(2):
                if lt + j >= nlt:
                    break
                lj, Lj = ltiles[lt + j]
                for nt in range(2):
                    nc.tensor.transpose(
                        mtp[0:Lj, j * 256 + nt * 128:j * 256 + (nt + 1) * 128],
                        mask_sb[nt][:, lj:lj + Lj], ident)
            npair = min(2, nlt - lt)
            # 0/1 mask from the additive-mask transpose: (x >= -0.5) as bf16.
            # On DVE (not ACT): ACT is the bottleneck engine, DVE has slack.
            nc.vector.tensor_scalar(m01T[:, lt:lt + npair, :],
                                    mtp[:, 0:npair * 256], -0.5, None,
                                    mybir.AluOpType.is_ge)
        # k tile [L, (h d)] -> transpose -> [hd, L] -> fold heads to base 0
        kin = sb_k.tile([128, 128], FP32, tag=f"kin_{tagp}")
        srck = bass.AP(tensor=kd.tensor,
                       offset=kd.offset + (b * H * LTOT + l0) * D,
                       ap=[[D, L], [LTOT * D, H], [1, D]])
        nc.sync.dma_start(out=kin[0:L, :], in_=srck)
        kt_ps = loop_ps()
        nc.tensor.transpose(kt_ps[:, 0:L], kin[0:L, :], ident[0:L, 0:L])
        ktf = sb_k.tile([128, 128], MM_DT, tag=f"ktf_{tagp}")
        nc.vector.tensor_copy(ktf[:, 0:L], kt_ps[:, 0:L])

        # v in 32-wide bands: cols 32h+[0:16] = v_h, col 32h+16 = ones,
        # rest zero (zeros make the unused PSUM rows exact zeros)
        v32 = sb_k.tile([128, 128], FP32, tag=f"v32_{tagp}")
        srcv = bass.AP(tensor=vd.tensor, offset=vd.offset + (b * H * LTOT + l0) * D,
                       ap=[[D, L], [LTOT * D, H], [1, D]])
        nc.sync.dma_start(out=v32[0:L, :], in_=srcv)
        vaug = vaug_slots[tagp][lt % 3]
        dstv = bass.AP(tensor=vaug.tensor, offset=vaug.offset,
                       ap=[[vaug.ap[0][0], L], [32, H], [1, D]])
        srcv2 = bass.AP(tensor=v32.tensor, offset=v32.offset,
                        ap=[[v32.ap[0][0], L], [16, H], [1, D]])
        nc.vector.tensor_copy(dstv, srcv2)

        # scores: K=128 against stacked kT; block-diagonal qtz isolates heads
        u = sb_u.tile([128, H, 256], BF16, tag=f"u_{tagp}")
        for p in range(2):
            sc_ps = ps_score.tile([128, 1024], FP32, tag="sc_ps")
            for j in range(2):
                nc.tensor.matmul(sc_ps[0:L, j * 512:(j + 1) * 512],
                                 ktf[:, 0:L],
                                 qtz[:, (4 * p + 2 * j) * 256:
                                     (4 * p + 2 * j + 2) * 256],
                                 start=True, stop=True)
            nc.scalar.activation(u[0:L, 4 * p:4 * p + 4, :], sc_ps[0:L, :],
                                 AF.Exp, scale=0.25)
        # mask (broadcast one tile across all 8 heads)
        mslice = m01T[0:L, lt, :]
        mb = bass.AP(tensor=mslice.tensor, offset=mslice.offset,
                     ap=[mslice.ap[0], [0, H], [1, 256]])
        nc.vector.tensor_mul(u[0:L], u[0:L], mb)

        # attention output (+ denominator row), accumulated over l tiles
        for h in range(H):
            nc.tensor.matmul(att_ps[h // 4][32 * (h % 4):32 * (h % 4) + 32, :],
                             vaug[0:L, h * 32:(h + 1) * 32],
                             u[0:L, h, :],
                             start=(lt == 0), stop=(lt == nlt - 1),
                             tile_position=(0, 32 * (h % 4)),
                             skip_group_check=True)

    # copy att PSUM to SBUF (keeps att layout); per 32-row head block the
    # denominator sits at local row 16 -- broadcast it to all rows with one
    # stream_shuffle, reciprocal, then normalize
    attc = sb_misc.tile([128, 512], MM_DT, tag=f"attc_{tagp}")
    for c in range(2):
        nc.vector.tensor_copy(attc[:, c * 256:(c + 1) * 256], att_ps[c])
    dshuf = sb_misc.tile([128, 512], MM_DT, tag=f"dshuf_{tagp}")
    nc.vector.stream_shuffle(dshuf.bitcast(FP32), attc.bitcast(FP32), [16] * 32)
    den_r = sb_misc.tile([128, 512], MM_DT, tag=f"denr_{tagp}")
    nc.vector.reciprocal(den_r, dshuf)
    mhc = sb_misc.tile([128, 512], MM_DT, tag=f"mhc_{tagp}")
    nc.gpsimd.tensor_mul(mhc, attc, den_r)
    return mhc


# ------------------------------------------------------------- entry point
_NC_CACHE = None


def kernel(**inputs):
    global _NC_CACHE
    from concourse.bass_utils import run_bass_kernel_spmd

    if _NC_CACHE is None:
        _NC_CACHE = build_nc()
    nc = _NC_CACHE
    res = run_bass_kernel_spmd(nc, _in_maps(inputs), core_ids=list(range(NCORES)))
    return np.concatenate([res.results[c]["out"] for c in range(NCORES)], axis=0)


def _in_maps(inputs):
    arrs = {
        "eln": "encoded_last_node", "load": "load", "solm": "sols_mask_pomo",
        "ninf": "ninf_mask", "k": "k", "v": "v", "k_s": "k_s", "v_s": "v_s",
        "shk": "single_head_key", "wq": "Wq_last", "wc": "W_combine",
    }
    data = {n: np.ascontiguousarray(np.asarray(inputs[key], np.float32))
            for n, key in arrs.items()}
    in_maps = []
    for c in range(NCORES):
        s = slice(c * BLOC, (c + 1) * BLOC)
        in_maps.append({n: (a[s] if n not in ("wq", "wc") else a)
                        for n, a in data.items()})
    return in_maps


def bench(inputs, iters=6):
    """Measure per-launch hardware execution time.

    A single launch through the axon PJRT tunnel is dominated by a fixed
    ~70-90ms client<->terminal round trip (a trivial 2-DMA kernel measures
    the same wall time as this kernel), so single-launch wall clock says
    nothing about the device. Launches pipeline perfectly through the
    tunnel (K chained launches ~= 1 RTT + K * device_time), so the slope
    between two chain lengths cancels the RTT and yields the on-device
    time per launch. Device-side serialization is forced by donating
    launch i's output as launch i+1's output buffer (the kernel fully
    overwrites its output, so correctness is unaffected).
    """
    import time
    import jax
    import concourse.mybir as mb
    from concourse import bass2jax
    from jax.experimental.shard_map import shard_map
    from jax.sharding import Mesh, NamedSharding, PartitionSpec

    global _NC_CACHE
    if _NC_CACHE is None:
        _NC_CACHE = build_nc()
    nc = _NC_CACHE
    bass2jax.install_neuronx_cc_hook()

    partition_name = nc.partition_id_tensor.name if nc.partition_id_tensor else None
    in_names, out_names, out_avals, zero_outs = [], [], [], []
    for alloc in nc.m.functions[0].allocations:
        if not isinstance(alloc, mb.MemoryLocationSet):
            continue
        name = alloc.memorylocations[0].name
        if alloc.kind == "ExternalInput":
            if name != partition_name:
                in_names.append(name)
        elif alloc.kind == "ExternalOutput":
            shape = tuple(alloc.tensor_shape)
            dtype = mb.dt.np(alloc.dtype)
            out_names.append(name)
            out_avals.append(jax.core.ShapedArray(shape, dtype))
            zero_outs.append(np.zeros((NCORES * shape[0], *shape[1:]), dtype))
    n_params = len(in_names)
    n_outs = len(out_avals)
    all_names = in_names + out_names + ([partition_name] if partition_name else [])
    donate = tuple(range(n_params, n_params + n_outs))

    def _body(*args):
        operands = list(args)
        if partition_name is not None:
            operands.append(bass2jax.partition_id_tensor())
        return tuple(bass2jax._bass_exec_p.bind(
            *operands, out_avals=tuple(out_avals), in_names=tuple(all_names),
            out_names=tuple(out_names), lowering_input_output_aliases=(),
            sim_require_finite=True, sim_require_nnan=True, nc=nc))

    devices = jax.devices()[:NCORES]
    mesh = Mesh(np.asarray(devices), ("core",))
    sharded = jax.jit(
        shard_map(_body, mesh=mesh,
                  in_specs=(PartitionSpec("core"),) * (n_params + n_outs),
                  out_specs=(PartitionSpec("core"),) * n_outs, check_rep=False),
        donate_argnums=donate, keep_unused=True)

    in_maps = _in_maps(inputs)
    concat_in = [np.concatenate([np.asarray(in_maps[c][nm]) for c in range(NCORES)],
                                axis=0) for nm in in_names]
    sh = NamedSharding(mesh, PartitionSpec("core"))
    dev_in = [jax.device_put(a, sh) for a in concat_in]

    def chain(k, outs):
        t0 = time.perf_counter()
        for _ in range(k):
            outs = list(sharded(*dev_in, *outs))
        jax.block_until_ready(outs)
        return time.perf_counter() - t0, outs

    outs = [jax.device_put(z, sh) for z in zero_outs]
    _, outs = chain(2, outs)  # warmup (compile + HAM)
    k_lo, k_hi = 8, 104
    best = None
    for rep in range(3):
        t_lo, outs = chain(k_lo, outs)
        t_hi, outs = chain(k_hi, outs)
        per_launch = (t_hi - t_lo) / (k_hi - k_lo)
        print(f"  chain timing rep{rep}: T({k_lo})={t_lo*1e3:.2f}ms "
              f"T({k_hi})={t_hi*1e3:.2f}ms -> {per_launch*1e6:.0f} us/launch")
        if per_launch > 0 and (best is None or per_launch < best):
            best = per_launch
    return int(best * 1e9)


if __name__ == "__main__":
    build_nc()
    print("build ok")

